# revision 2
# baseline (speedup 1.0000x reference)
"""Trainium2 Bass kernel for nn_BinsCombinerLayer (histogram_binning).

Reference computation:
    per_set_cumsum = cumsum(inputs * centroids, axis=1)   # [S, B]
    out = sum(per_set_cumsum, axis=0) / S                 # [B]

Math: cumsum (over bins) is linear, so it commutes with the sum over sets
and with the cross-core all-reduce:
    out = cumsum_b( sum_s inputs[s,b] * centroids[s,b] ) / S

Sharding (8 cores, data-parallel over the set axis):
  - each core streams its [1024, 4096] shard of inputs/centroids,
    computes prod = inputs*centroids on the Vector engine,
    reduces over the set axis with a ones-vector matmul on the Tensor
    engine (PSUM accumulation across row tiles),
  - scales by 1/S, runs the 4096-wide cumsum locally (DVE scan),
  - AllReduce(add) of the per-core cumsummed partials produces the final
    answer on every core.
"""

import sys

sys.path.insert(0, "/opt/trn_rl_repo")

import numpy as np

N_CORES = 8
S, B = 8192, 4096
S_SHARD = S // N_CORES  # 1024 rows per core
P = 128                 # partitions per row tile
R = S_SHARD // P        # 8 row tiles per core
CHUNK = 512             # matmul moving free dim (one PSUM bank)
NCHUNK = B // CHUNK     # 8

_CACHE = {}


def _build():
    import concourse.bacc as bacc
    import concourse.tile as tile
    import concourse.mybir as mybir

    f32 = mybir.dt.float32
    nc = bacc.Bacc(
        "TRN2", target_bir_lowering=False, debug=False, num_devices=N_CORES
    )
    inp = nc.dram_tensor("inputs", [S_SHARD, B], f32, kind="ExternalInput").ap()
    cen = nc.dram_tensor("centroids", [S_SHARD, B], f32, kind="ExternalInput").ap()
    out = nc.dram_tensor("out", [1, B], f32, kind="ExternalOutput").ap()

    with tile.TileContext(nc) as tc:
        with (
            tc.tile_pool(name="io", bufs=3) as io,
            tc.tile_pool(name="work", bufs=2) as work,
            tc.tile_pool(name="small", bufs=1) as small,
            tc.tile_pool(name="psum", bufs=1, space="PSUM") as psum,
            tc.tile_pool(name="dram", bufs=1, space="DRAM") as dram,
        ):
            ones = small.tile([P, 1], f32, tag="ones")
            nc.vector.memset(ones[:], 1.0)

            q_ps = [
                psum.tile([1, CHUNK], f32, tag=f"q{j}", name=f"q_ps{j}")
                for j in range(NCHUNK)
            ]

            for i in range(R):
                in_t = io.tile([P, B], f32, tag="in")
                ce_t = io.tile([P, B], f32, tag="cen")
                nc.sync.dma_start(in_t[:], inp[i * P : (i + 1) * P, :])
                nc.sync.dma_start(ce_t[:], cen[i * P : (i + 1) * P, :])
                prod = work.tile([P, B], f32, tag="prod")
                nc.vector.tensor_mul(prod[:], in_t[:], ce_t[:])
                for j in range(NCHUNK):
                    nc.tensor.matmul(
                        q_ps[j][:],
                        ones[:],
                        prod[:, j * CHUNK : (j + 1) * CHUNK],
                        start=(i == 0),
                        stop=(i == R - 1),
                    )

            # Scale by 1/S while copying PSUM -> SBUF.
            q_sb = small.tile([1, B], f32, tag="q_sb")
            for j in range(NCHUNK):
                nc.scalar.mul(
                    q_sb[0:1, j * CHUNK : (j + 1) * CHUNK], q_ps[j][:], 1.0 / S
                )

            # Inclusive prefix sum along the bin axis.
            zeros = small.tile([1, B], f32, tag="zeros")
            nc.vector.memset(zeros[:], 0.0)
            cum = small.tile([1, B], f32, tag="cum")
            nc.vector.tensor_tensor_scan(
                cum[:],
                q_sb[:],
                zeros[:],
                0.0,
                op0=mybir.AluOpType.add,
                op1=mybir.AluOpType.add,
            )

            # AllReduce of the cumsummed partials == final output.
            cc_in = dram.tile([1, B], f32, tag="cc_in")
            cc_out = dram.tile([1, B], f32, tag="cc_out")
            nc.sync.dma_start(cc_in[:], cum[:])
            nc.gpsimd.collective_compute(
                "AllReduce",
                mybir.AluOpType.add,
                replica_groups=[list(range(N_CORES))],
                ins=[cc_in.opt()],
                outs=[cc_out.opt()],
            )
            nc.sync.dma_start(out[:], cc_out[:])

    nc.compile()
    return nc


def _get_nc():
    if "nc" not in _CACHE:
        _CACHE["nc"] = _build()
    return _CACHE["nc"]


def kernel(inputs: np.ndarray, centroids: np.ndarray, **run_kwargs):
    from concourse.bass_utils import run_bass_kernel_spmd

    inputs = np.asarray(inputs, dtype=np.float32)
    centroids = np.asarray(centroids, dtype=np.float32)
    assert inputs.shape == (S, B) and centroids.shape == (S, B)

    nc = _get_nc()
    in_maps = [
        {
            "inputs": np.ascontiguousarray(inputs[c * S_SHARD : (c + 1) * S_SHARD]),
            "centroids": np.ascontiguousarray(
                centroids[c * S_SHARD : (c + 1) * S_SHARD]
            ),
        }
        for c in range(N_CORES)
    ]
    res = run_bass_kernel_spmd(nc, in_maps, core_ids=list(range(N_CORES)), **run_kwargs)
    out = res.results[0]["out"].reshape(B).astype(np.float32, copy=False)
    if run_kwargs:
        _CACHE["last_result"] = res
    return out


# revision 8
# speedup vs baseline: 1.0902x; 1.0902x over previous
"""Trainium2 Bass kernel for nn_BinsCombinerLayer (histogram_binning).

Reference computation:
    per_set_cumsum = cumsum(inputs * centroids, axis=1)   # [S, B]
    out = sum(per_set_cumsum, axis=0) / S                 # [B]

Math: cumsum (over bins) is linear, so it commutes with the sum over sets
and with the cross-core all-reduce:
    out = cumsum_b( sum_s inputs[s,b] * centroids[s,b] ) / S

Sharding (8 cores, data-parallel over the set axis):
  - each core streams its [1024, 4096] shard of inputs/centroids,
    computes prod = inputs*centroids on the Vector engine; row pairs are
    summed on DVE first so the fp32 ones-vector matmul reduction on the
    Tensor engine (PSUM accumulation) runs at half the matmul count,
  - the 4096-wide partial is reshaped to [128, 32], scanned in parallel
    per partition, cross-partition offsets come from a strictly-lower
    triangular ones matmul, scale by 1/S is fused into the offset add,
  - AllReduce(add) of the per-core cumsummed partials produces the final
    answer on every core.
"""

import sys

sys.path.insert(0, "/opt/trn_rl_repo")

import numpy as np

N_CORES = 8
S, B = 8192, 4096
S_SHARD = S // N_CORES  # 1024 rows per core
P = 128                 # partitions per row tile
R = S_SHARD // P        # 8 row tiles per core
NPAIR = R // 2          # 4 row-tile pairs
W = 2048                # free-dim tile width
NH = B // W             # 2 halves
CHUNK = 512             # matmul moving free dim (one PSUM bank)
NCHUNK = B // CHUNK     # 8
SCAN_F = B // P         # 32 bins per partition in the scan layout

_CACHE = {}


def _build():
    import concourse.bacc as bacc
    import concourse.tile as tile
    import concourse.mybir as mybir

    f32 = mybir.dt.float32
    add = mybir.AluOpType.add
    nc = bacc.Bacc(
        "TRN2", target_bir_lowering=False, debug=False, num_devices=N_CORES
    )
    inp = nc.dram_tensor("inputs", [S_SHARD, B], f32, kind="ExternalInput").ap()
    cen = nc.dram_tensor("centroids", [S_SHARD, B], f32, kind="ExternalInput").ap()
    out = nc.dram_tensor("out", [1, B], f32, kind="ExternalOutput").ap()

    with tile.TileContext(nc) as tc:
        with (
            tc.tile_pool(name="io", bufs=6) as io,
            tc.tile_pool(name="work", bufs=3) as work,
            tc.tile_pool(name="small", bufs=1) as small,
            tc.tile_pool(name="psum", bufs=1, space="PSUM") as psum,
            tc.tile_pool(name="dram", bufs=1, space="DRAM") as dram,
        ):
            ones = small.tile([P, 1], f32, tag="ones")
            nc.vector.memset(ones[:], 1.0)

            # mask[k, m] = 1 if k < m else 0 (same recipe as
            # masks.make_upper_triangular with diag=False).
            mask = small.tile([P, P], f32, tag="mask")
            nc.gpsimd.memset(mask[:], 0.0)
            nc.gpsimd.affine_select(
                out=mask[:],
                in_=mask[:],
                compare_op=mybir.AluOpType.is_ge,
                fill=1.0,
                base=0,
                pattern=[[-1, P]],
                channel_multiplier=1,
            )

            zeros32 = small.tile([P, SCAN_F], f32, tag="zeros32")
            nc.vector.memset(zeros32[:], 0.0)

            # One PSUM tensor spanning all 8 banks: bank j = q[j*512:(j+1)*512].
            psum_q = psum.tile([1, NCHUNK, CHUNK], f32, tag="psq")

            for k in range(NPAIR):
                for h in range(NH):
                    ia = io.tile([P, W], f32, tag="in", name=f"ia{k}{h}")
                    ca = io.tile([P, W], f32, tag="cen", name=f"ca{k}{h}")
                    ib = io.tile([P, W], f32, tag="in", name=f"ib{k}{h}")
                    cb = io.tile([P, W], f32, tag="cen", name=f"cb{k}{h}")
                    r0, r1 = 2 * k * P, (2 * k + 1) * P
                    nc.sync.dma_start(ia[:], inp[r0 : r0 + P, h * W : (h + 1) * W])
                    nc.sync.dma_start(ca[:], cen[r0 : r0 + P, h * W : (h + 1) * W])
                    nc.sync.dma_start(ib[:], inp[r1 : r1 + P, h * W : (h + 1) * W])
                    nc.sync.dma_start(cb[:], cen[r1 : r1 + P, h * W : (h + 1) * W])
                    pa = work.tile([P, W], f32, tag="pa", name=f"pa{k}{h}")
                    pb = work.tile([P, W], f32, tag="pb", name=f"pb{k}{h}")
                    nc.vector.tensor_mul(pa[:], ia[:], ca[:])
                    nc.vector.tensor_mul(pb[:], ib[:], cb[:])
                    nc.vector.tensor_add(pa[:], pa[:], pb[:])
                    for jj in range(W // CHUNK):
                        j = h * (W // CHUNK) + jj
                        nc.tensor.matmul(
                            psum_q[0:1, j, :],
                            ones[:],
                            pa[:, jj * CHUNK : (jj + 1) * CHUNK],
                            start=(k == 0),
                            stop=(k == NPAIR - 1),
                        )

            # PSUM -> SBUF copy with the 1/S scale folded in, then an
            # SBUF->SBUF scatter DMA into the [128, 32] partition-major
            # scan layout.
            q_sb = small.tile([1, B], f32, tag="q_sb")
            nc.scalar.mul(
                q_sb[:].rearrange("p (a b) -> p a b", a=NCHUNK),
                psum_q[0:1, :, :],
                1.0 / S,
            )
            q_resh = small.tile([P, SCAN_F], f32, tag="q_resh")
            nc.sync.dma_start(q_resh[:], q_sb[:])

            # Per-partition inclusive scan over 32 bins.
            scan_t = small.tile([P, SCAN_F], f32, tag="scan_t")
            nc.vector.tensor_tensor_scan(
                scan_t[:], q_resh[:], zeros32[:], 0.0, op0=add, op1=add
            )

            # Cross-partition exclusive-scan of per-partition totals.
            # Same tag as psum_q: all 8 PSUM banks are taken by psum_q, so the
            # offs matmul reuses that slot after psum_q is drained to DRAM.
            offs_ps = psum.tile([P, 1], f32, tag="psq", name="offs_ps")
            nc.tensor.matmul(
                offs_ps[:], mask[:], scan_t[:, SCAN_F - 1 : SCAN_F],
                start=True, stop=True,
            )

            # cum = scan + offs (inputs already carry the 1/S scale).
            cc_src = small.tile([P, SCAN_F], f32, tag="cc_src")
            nc.vector.tensor_scalar(
                cc_src[:],
                scan_t[:],
                offs_ps[:, 0:1],
                None,
                op0=add,
            )

            # AllReduce of the cumsummed partials == final output.
            cc_in = dram.tile([P, SCAN_F], f32, tag="cc_in")
            cc_out = dram.tile([P, SCAN_F], f32, tag="cc_out")
            nc.sync.dma_start(cc_in[:], cc_src[:])
            nc.gpsimd.collective_compute(
                "AllReduce",
                add,
                replica_groups=[list(range(N_CORES))],
                ins=[cc_in.opt()],
                outs=[cc_out.opt()],
            )
            nc.sync.dma_start(out[:], cc_out[:])

    nc.compile()
    return nc


def _get_nc():
    if "nc" not in _CACHE:
        _CACHE["nc"] = _build()
    return _CACHE["nc"]


def kernel(inputs: np.ndarray, centroids: np.ndarray, **run_kwargs):
    from concourse.bass_utils import run_bass_kernel_spmd

    inputs = np.asarray(inputs, dtype=np.float32)
    centroids = np.asarray(centroids, dtype=np.float32)
    assert inputs.shape == (S, B) and centroids.shape == (S, B)

    nc = _get_nc()
    in_maps = [
        {
            "inputs": np.ascontiguousarray(inputs[c * S_SHARD : (c + 1) * S_SHARD]),
            "centroids": np.ascontiguousarray(
                centroids[c * S_SHARD : (c + 1) * S_SHARD]
            ),
        }
        for c in range(N_CORES)
    ]
    res = run_bass_kernel_spmd(nc, in_maps, core_ids=list(range(N_CORES)), **run_kwargs)
    out = res.results[0]["out"].reshape(B).astype(np.float32, copy=False)
    if run_kwargs:
        _CACHE["last_result"] = res
    return out
